# revision 12
# baseline (speedup 1.0000x reference)
"""BetaTCVAE loss kernel for 8 TRN2 NeuronCores (Bass/Tile).

Math
----
reference:  out = (BETA-1)*tc + sum(kl)
  lp[i,j,d] = -0.5*((z_i - m_j)^2 * exp(-lv_j) + lv_j + LOG2PI)   (per dim d)
  log_qz_product[i] = sum_d logsumexp_j lp[i,j,d]
  log_qz[i]         = logsumexp_j sum_d lp[i,j,d]
  tc = mean_i(log_qz - log_qz_product)

Decomposition used here (per core, rows i sharded 256/core):
  With s^2 = exp(-lv)/2 define the shifted exponent
      t[i,j,d] = -s2*z^2 + wm*z + e,
      wm = 2*s2*m,  e = -0.5*(wm*m + lv) - 0.5*ln2
  so that t = lp + (LOG2PI - ln2)/2 per dim.  Then
      A[i,d]   = sum_j exp(t[i,j,d])       ->  P[i] = sum_d ln A[i,d]
      S[i,j]   = sum_d t[i,j,d]            ->  lqz[i] = logsumexp_j S[i,j]
      contrib[i] = lqz[i] - P[i]           (shift constants cancel)
  tc = mean_i contrib.

Mapping to engines:
  * t for one latent dim d is a rank-3 bilinear form: one [3 x 128] lhsT
    (-z^2, z, 1 per row-i) against a [3 x 512] rhs (s2, wm, e per col-j)
    -> 4 matmuls fill a [128 x 2048] PSUM tile on the Tensor engine
    (PSUM-write-port bound at ~427ns per matmul).
  * A[i,d]: one Activation Exp instruction over the PSUM tile, j-reduce
    split between the ACT accumulator and DVE tensor_reduce to balance
    engine load.
  * S via three 64-contraction bf16 matmuls (same H rows), classic
    max-shifted logsumexp epilogue; runs before the hot loop so the
    tail stays short.
  * Final: out = (BETA-1)*(T_sum/B) + KL_sum  (host side).
"""

import math
import sys

import numpy as np

if "/opt/trn_rl_repo" not in sys.path:
    sys.path.insert(0, "/opt/trn_rl_repo")

import concourse.bacc as bacc
import concourse.tile as tile
from concourse import mybir
from concourse.bass_utils import run_bass_kernel_spmd
from concourse.masks import make_identity

B, D, M = 2048, 64, 8
BL = B // M          # 256 local rows
NJT = B // 128       # 16 natural j-tiles
DCH = 8              # latent dims per H chunk
NCH = D // DCH       # 8 chunks
F32 = mybir.dt.float32
BF16 = mybir.dt.bfloat16
LN2 = math.log(2.0)
BETA = 6.0

A = mybir.AluOpType
AF = mybir.ActivationFunctionType
AX = mybir.AxisListType


def _body(tc):
    nc = tc.nc
    kl_ext = nc.dram_tensor("kl", [BL, D], F32, kind="ExternalInput").ap()
    zm_ext = nc.dram_tensor("z_mean", [B, D], F32, kind="ExternalInput").ap()
    zlv_ext = nc.dram_tensor("z_logvar", [B, D], F32, kind="ExternalInput").ap()
    zs_ext = nc.dram_tensor("z_sampled", [BL, D], F32, kind="ExternalInput").ap()
    out_ext = nc.dram_tensor("out", [1, 2], F32, kind="ExternalOutput").ap()

    with (
        tc.tile_pool(name="cst", bufs=1) as cst,
        tc.tile_pool(name="mats", bufs=1) as mats,
        tc.tile_pool(name="ld", bufs=6) as ld,
    ):
        ident = cst.tile([128, 128], F32, tag="ident")
        make_identity(nc, ident)
        ones = cst.tile([128, 1], F32, tag="ones")
        nc.vector.memset(ones, 1.0)
        bias_l2 = cst.tile([128, 1], F32, tag="bias_l2")
        nc.gpsimd.memset(bias_l2, math.log(0.5))
        ones_rep = cst.tile([128, B], BF16, tag="ones_rep")
        nc.gpsimd.memset(ones_rep, 1.0)

        # ---- load + transpose: zlv first (prep chain starts on it) ----
        m_t = mats.tile([64, B], F32, tag="m_t")
        lv_t = mats.tile([64, B], F32, tag="lv_t")
        z_t = mats.tile([64, BL], F32, tag="z_t")
        with tc.tile_pool(name="pst", bufs=4, space="PSUM") as pst:
            for t in range(NJT):
                nat = ld.tile([128, D], F32, tag="nat")
                nc.sync.dma_start(out=nat, in_=zlv_ext[t * 128:(t + 1) * 128, :])
                ps = pst.tile([64, 128], F32, tag="tp")
                nc.tensor.transpose(ps, nat, ident)
                nc.vector.tensor_copy(out=lv_t[0:64, t * 128:(t + 1) * 128], in_=ps)
            for t in range(2):
                nat = ld.tile([128, D], F32, tag="nat")
                nc.sync.dma_start(out=nat, in_=zs_ext[t * 128:(t + 1) * 128, :])
                ps = pst.tile([64, 128], F32, tag="tp")
                nc.tensor.transpose(ps, nat, ident)
                nc.vector.tensor_copy(out=z_t[0:64, t * 128:(t + 1) * 128], in_=ps)
            for t in range(NJT):
                nat = ld.tile([128, D], F32, tag="nat")
                nc.sync.dma_start(out=nat, in_=zm_ext[t * 128:(t + 1) * 128, :])
                ps = pst.tile([64, 128], F32, tag="tp")
                nc.tensor.transpose(ps, nat, ident)
                nc.vector.tensor_copy(out=m_t[0:64, t * 128:(t + 1) * 128], in_=ps)

        # ---- prep H rows (j side, [64 d, 2048 j]) ----
        # s2 = exp(-lv)/2; wm = 2*s2*m; e = -0.5*(wm*m + lv) - ln2/2
        # ACT handles the scale-only ops (immediate scale, no AP reads),
        # DVE the two-input ones.
        hrow0 = mats.tile([64, B], BF16, tag="hrow0")
        nc.scalar.activation(out=hrow0[0:64, :], in_=lv_t[0:64, :], func=AF.Exp,
                             bias=bias_l2[0:64, :], scale=-1.0)
        lvh_t = mats.tile([64, B], F32, tag="lvh_t")
        nc.scalar.activation(out=lvh_t[0:64, :], in_=lv_t[0:64, :], func=AF.Copy,
                             bias=-0.5 * LN2, scale=-0.5)
        m2_t = mats.tile([64, B], F32, tag="m2_t")
        nc.scalar.activation(out=m2_t[0:64, :], in_=m_t[0:64, :], func=AF.Copy,
                             bias=0.0, scale=2.0)
        mneg_t = mats.tile([64, B], F32, tag="mneg_t")
        nc.scalar.activation(out=mneg_t[0:64, :], in_=m_t[0:64, :], func=AF.Copy,
                             bias=0.0, scale=-0.5)
        hrow1 = mats.tile([64, B], BF16, tag="hrow1")
        nc.vector.tensor_mul(out=hrow1[0:64, :], in0=hrow0[0:64, :],
                             in1=m2_t[0:64, :])
        e1_t = mats.tile([64, B], F32, tag="e1_t")
        nc.vector.tensor_mul(out=e1_t[0:64, :], in0=hrow1[0:64, :],
                             in1=mneg_t[0:64, :])
        hrow2 = mats.tile([64, B], BF16, tag="hrow2")
        nc.vector.tensor_add(out=hrow2[0:64, :], in0=e1_t[0:64, :],
                             in1=lvh_t[0:64, :])

        # ---- prep G rows (i side) ----
        z2f = mats.tile([64, BL], F32, tag="z2f")
        nc.vector.tensor_mul(out=z2f[0:64, :], in0=z_t[0:64, :], in1=z_t[0:64, :])
        nz2_b = mats.tile([64, BL], BF16, tag="nz2_b")
        nc.vector.tensor_scalar(out=nz2_b[0:64, :], in0=z2f[0:64, :],
                                scalar1=-1.0, scalar2=None, op0=A.mult)
        z_b = mats.tile([64, BL], BF16, tag="z_b")
        nc.vector.tensor_copy(out=z_b[0:64, :], in_=z_t[0:64, :])
        ones_b = mats.tile([64, BL], BF16, tag="ones_b")
        nc.gpsimd.memset(ones_b, 1.0)

        # G_mega [3, 64*256]: per-(d,it) lhsT slices, d-major from [64,256] rows
        g_mega = mats.tile([3, D * BL], BF16, tag="g_mega")
        nc.sync.dma_start(out=g_mega[0:1, :], in_=nz2_b[0:64, :])
        nc.sync.dma_start(out=g_mega[1:2, :], in_=z_b[0:64, :])
        nc.sync.dma_start(out=g_mega[2:3, :], in_=ones_b[0:64, :])

        # ---- S matmuls + logsumexp (pre-Ln part), before the hot loop ----
        nmxs, esums = [], []
        with (
            tc.tile_pool(name="psp", bufs=1, space="PSUM") as psp,
            tc.tile_pool(name="scr", bufs=2) as scr,
        ):
            for it in range(2):
                isl = slice(it * 128, (it + 1) * 128)
                sps = []
                for jb in range(4):
                    jsl = slice(jb * 512, (jb + 1) * 512)
                    sp = psp.tile([128, 512], F32, tag=f"sp{jb}")
                    nc.tensor.matmul(sp, lhsT=nz2_b[0:64, isl], rhs=hrow0[0:64, jsl],
                                     start=True, stop=False)
                    nc.tensor.matmul(sp, lhsT=z_b[0:64, isl], rhs=hrow1[0:64, jsl],
                                     start=False, stop=False)
                    nc.tensor.matmul(sp, lhsT=ones_b[0:64, 0:128], rhs=hrow2[0:64, jsl],
                                     start=False, stop=True)
                    sps.append(sp)
                mx4 = mats.tile([128, 4], F32, tag="mx4", bufs=2)
                for jb in range(4):
                    nc.vector.tensor_reduce(out=mx4[:, jb:jb + 1], in_=sps[jb],
                                            axis=AX.X, op=A.max)
                nmx = mats.tile([128, 1], F32, tag="nmx", bufs=2)
                nc.vector.tensor_reduce(out=nmx, in_=mx4, axis=AX.X, op=A.max,
                                        negate=True)
                es4 = mats.tile([128, 4], F32, tag="es4", bufs=2)
                for jb in range(4):
                    sc = scr.tile([128, 512], BF16, tag="sc")
                    nc.scalar.activation(out=sc, in_=sps[jb], func=AF.Exp,
                                         bias=nmx, scale=1.0,
                                         accum_out=es4[:, jb:jb + 1])
                esum = mats.tile([128, 1], F32, tag="esum", bufs=2)
                nc.vector.tensor_reduce(out=esum, in_=es4, axis=AX.X, op=A.add)
                nmxs.append(nmx)
                esums.append(esum)

        # A[i,d] accumulators, one per i-tile (+ bf16 probe staging col)
        a_mat0 = mats.tile([128, D], F32, tag="a_mat0")
        a_mat1 = mats.tile([128, D], F32, tag="a_mat1")
        a_mat = [a_mat0, a_mat1]
        abp = mats.tile([128, 2], BF16, tag="abp")

        # ---- A hot loop: PE bilinear form -> ACT Exp -> j-reduce ----
        with (
            tc.tile_pool(name="hp", bufs=2) as hp,
            tc.tile_pool(name="pa", bufs=2, space="PSUM") as pa,
            tc.tile_pool(name="ep", bufs=3) as ep,
        ):
            for c in range(NCH):
                hch = hp.tile([3, DCH * B], BF16, tag="hch")
                dsl = slice(c * DCH, (c + 1) * DCH)
                nc.sync.dma_start(out=hch[0:1, :], in_=hrow0[dsl, :])
                nc.sync.dma_start(out=hch[1:2, :], in_=hrow1[dsl, :])
                nc.sync.dma_start(out=hch[2:3, :], in_=hrow2[dsl, :])
                for dd in range(DCH):
                    d = c * DCH + dd
                    for it in range(2):
                        tp = pa.tile([128, B], F32, tag="tp")
                        lhs = g_mega[0:3, d * BL + it * 128: d * BL + (it + 1) * 128]
                        for jb in range(4):
                            nc.tensor.matmul(
                                tp[:, jb * 512:(jb + 1) * 512], lhsT=lhs,
                                rhs=hch[0:3, dd * B + jb * 512: dd * B + (jb + 1) * 512],
                                start=True, stop=True)
                        e_t = ep.tile([128, B], BF16, tag="e")
                        if dd == 0:
                            # ~1/8 of reduces ride the ACT accumulator
                            nc.scalar.activation(out=e_t, in_=tp, func=AF.Exp,
                                                 bias=0.0, scale=1.0,
                                                 accum_out=a_mat[it][:, d:d + 1])
                        else:
                            nc.scalar.activation(out=e_t, in_=tp, func=AF.Exp,
                                                 bias=0.0, scale=1.0)
                            if dd == 1:
                                # probe: bf16-out reduce (maybe 2x mode)
                                with nc.allow_low_precision(reason="2x probe"):
                                    nc.vector.tensor_reduce(out=abp[:, it:it + 1],
                                                            in_=e_t, axis=AX.X,
                                                            op=A.add)
                                nc.vector.tensor_copy(
                                    out=a_mat[it][:, d:d + 1], in_=abp[:, it:it + 1])
                            elif dd == 2:
                                # probe: affine_mul_reduce with ones (maybe 2x)
                                e_s = ep.tile([128, B], BF16, tag="e_s")
                                nc.vector.affine_mul_reduce(
                                    out=e_s, accum_out=a_mat[it][:, d:d + 1],
                                    in0=e_t, in1=ones_rep, scale=1.0, bias=0.0)
                            else:
                                nc.vector.tensor_reduce(out=a_mat[it][:, d:d + 1],
                                                        in_=e_t, axis=AX.X, op=A.add)

        # ---- kl partial sum (off the critical path) ----
        ks2 = mats.tile([128, 2], F32, tag="ks2")
        for t in range(2):
            klt = ld.tile([128, D], F32, tag="klt", bufs=2)
            nc.sync.dma_start(out=klt, in_=kl_ext[t * 128:(t + 1) * 128, :])
            nc.vector.tensor_reduce(out=ks2[:, t:t + 1], in_=klt, axis=AX.X, op=A.add)
        kss = mats.tile([128, 1], F32, tag="kss")
        nc.vector.tensor_reduce(out=kss, in_=ks2, axis=AX.X, op=A.add)

        # ---- Ln epilogue + final scalars ----
        with tc.tile_pool(name="psm", bufs=2, space="PSUM") as psm:
            contrib = []
            for it in range(2):
                ln_a = mats.tile([128, D], F32, tag="ln_a", bufs=2)
                nc.scalar.activation(out=ln_a, in_=a_mat[it], func=AF.Ln,
                                     bias=0.0, scale=1.0)
                p_col = mats.tile([128, 1], F32, tag="p_col", bufs=2)
                nc.vector.tensor_reduce(out=p_col, in_=ln_a, axis=AX.X, op=A.add)
                lqz = mats.tile([128, 1], F32, tag="lqz", bufs=2)
                nc.scalar.activation(out=lqz, in_=esums[it], func=AF.Ln,
                                     bias=0.0, scale=1.0)
                mx = mats.tile([128, 1], F32, tag="mx", bufs=2)
                nc.vector.tensor_scalar(out=mx, in0=nmxs[it], scalar1=-1.0,
                                        scalar2=None, op0=A.mult)
                nc.vector.tensor_add(out=lqz, in0=lqz, in1=mx)
                ctr = mats.tile([128, 1], F32, tag="ctr", bufs=2)
                nc.vector.tensor_sub(out=ctr, in0=lqz, in1=p_col)
                contrib.append(ctr)

            fps = psm.tile([1, 2], F32, tag="fps")
            nc.tensor.matmul(fps[0:1, 0:1], lhsT=contrib[0], rhs=ones,
                             start=True, stop=False)
            nc.tensor.matmul(fps[0:1, 0:1], lhsT=contrib[1], rhs=ones,
                             start=False, stop=True)
            nc.tensor.matmul(fps[0:1, 1:2], lhsT=kss, rhs=ones,
                             start=True, stop=True)
            out_sb = mats.tile([1, 2], F32, tag="out_sb")
            nc.vector.tensor_copy(out=out_sb[0:1, :], in_=fps[0:1, :])
            nc.sync.dma_start(out=out_ext, in_=out_sb[0:1, :])


_NC_CACHE = {}


def _get_nc():
    if "nc" not in _NC_CACHE:
        nc = bacc.Bacc("TRN2", target_bir_lowering=False, debug=False,
                       num_devices=M)
        with tile.TileContext(nc) as tc:
            _body(tc)
        nc.compile()
        _NC_CACHE["nc"] = nc
    return _NC_CACHE["nc"]


def kernel(kl, z_mean, z_logvar, z_sampled, _trace=False, _tmpdir=None):
    kl = np.ascontiguousarray(kl, dtype=np.float32)
    z_mean = np.ascontiguousarray(z_mean, dtype=np.float32)
    z_logvar = np.ascontiguousarray(z_logvar, dtype=np.float32)
    z_sampled = np.ascontiguousarray(z_sampled, dtype=np.float32)
    nc = _get_nc()
    in_maps = []
    for c in range(M):
        sl = slice(c * BL, (c + 1) * BL)
        in_maps.append({
            "kl": np.ascontiguousarray(kl[sl]),
            "z_mean": z_mean,
            "z_logvar": z_logvar,
            "z_sampled": np.ascontiguousarray(z_sampled[sl]),
        })
    res = run_bass_kernel_spmd(nc, in_maps, list(range(M)), trace=_trace,
                               tmpdir=_tmpdir)
    t_sum = 0.0
    kl_sum = 0.0
    for c in range(M):
        o = res.results[c]["out"]
        t_sum += float(o[0, 0])
        kl_sum += float(o[0, 1])
    val = (BETA - 1.0) * (t_sum / B) + kl_sum
    out = np.float32(val)
    if _trace:
        return out, res
    return out
